# revision 28
# baseline (speedup 1.0000x reference)
"""BERT self-attention Bass/Tile kernel for 8 Trainium2 NeuronCores.

Problem: hidden [2, 2048, 768], 12 heads x 64 dim, additive mask [2,1,1,2048].
Sharding: batch x head-group. Core c handles batch b = c // 4 and global heads
3*(c%4) .. 3*(c%4)+2 (columns 192*(c%4) .. +192 of Wq/Wk/Wv).  Each core
computes its 3 heads' full attention locally; outputs are concatenated on the
host (no cross-device communication).

Host-side prep (part of sharding): X is passed pre-transposed and pre-cast to
fp16 ([768, 2048] per batch), and the weight slices are packed into their
on-chip fp16 layouts.  The device then runs pure matmul pipeline with no
PE transposes and no DVE casts:

  XT[f] [128, 2048] fp16  <- DMA (m-block pieces, f0-2 via SWDGE early)
  Q_T/K_T = W.T @ X_T     (heads 0/1 packed M=128; head-2 of Q and K merged
                           into one M=128 stationary [wq_h2|wk_h2])
  V[k, d] = X_T.T @ Wv    directly in [k, d] layout (XT chunk stationary);
                           bias via partition-broadcast tile on DVE
  scores_T[k,q] = K_T.T @ Q_T  (K=64 contraction; the two 64-row streams run
                                concurrently in the PE's row halves)
  probs = exp(scores/8) via ScalarE: ONE [128, 1024] PSUM tile and ONE exp
    per (k-chunk, step) covering both streams, so all four score matmuls are
    released by a single PSUM-free event (keeps pair concurrency intact under
    just-in-time recycling).
  V[kc] layout [V_h0|e|V_h1|e|V_h2|e] where e = exp(mask_k) column
  ctx_aug[q, 65] = probs_T.T @ V_aug  (col 64 = softmax denominator)
  out[q, d] = ctx[:, :64] * (1 / ctx[:, 64])   -> DMA to DRAM

The additive mask is folded into V: exp(s + m_k) = exp(s) * exp(m_k), so both
the numerator and the denominator column of V_aug are pre-scaled by exp(m_k).
When the mask is all zeros (the common case) that scale is skipped and the
denominator column is just memset to 1.
"""

import numpy as np

import concourse.bass as bass
import concourse.tile as tile
from concourse import bacc, mybir
from concourse.bass_utils import run_bass_kernel_spmd

F32 = mybir.dt.float32
F16 = mybir.dt.float16
EXP = mybir.ActivationFunctionType.Exp

S = 2048           # sequence length
DM = 768           # model dim
DH = 64            # head dim
NHL = 3            # local heads per core
FC = DM // 128     # 6 f-chunks (contraction for projections)
KC = S // 128      # 16 k-chunks
QB = 512           # q block width for score matmuls
NQB = S // QB      # 4 q blocks
# One group = one k-chunk and BOTH row-tiled streams in a single [128, 1024]
# PSUM tile (2 banks) drained by a single exp.
GROUPS = [(i, 1) for i in range(16)]


def _build_kernel(zero_mask: bool) -> bass.Bass:
    nc = bacc.Bacc()

    xt_d = nc.declare_dram_parameter("xt", [DM, S], F16, isOutput=False)
    w_d = nc.declare_dram_parameter("wall", [128, 3 * 6 * 192], F16,
                                    isOutput=False)
    qk2_d = nc.declare_dram_parameter("qk2", [128, 6 * 128], F16,
                                      isOutput=False)
    bq_d = nc.declare_dram_parameter("bq", [192], F32, isOutput=False)
    bk_d = nc.declare_dram_parameter("bk", [192], F32, isOutput=False)
    bv_d = nc.declare_dram_parameter("bv", [192], F32, isOutput=False)
    m_d = nc.declare_dram_parameter("mask", [S], F32, isOutput=False)
    out_d = nc.declare_dram_parameter("out", [S, 192], F32, isOutput=True)

    with tile.TileContext(nc) as tc:
        _attention(tc, xt_d, w_d, qk2_d, (bq_d, bk_d, bv_d), m_d, out_d,
                   zero_mask)
    nc.compile()
    return nc


def _attention(tc, xt_d, w_d, qk2_d, b_ds, m_d, out_d, zero_mask):
    nc = tc.nc

    const = tc.alloc_tile_pool(name="const", bufs=1)
    persist = tc.alloc_tile_pool(name="persist", bufs=1)
    probs_pool = tc.alloc_tile_pool(name="probs", bufs=58)
    small = tc.alloc_tile_pool(name="small", bufs=4)
    outp = tc.alloc_tile_pool(name="outp", bufs=1)
    ps = tc.alloc_tile_pool(name="ps", bufs=2, space="PSUM")

    # XT[f]: X.T rows 128f..128(f+1), fp16 [128, 2048], loaded in m-block
    # pieces so block 0 lands first.  f0-2 of block 0 go out on gpsimd whose
    # SWDGE issues at ~2.5us, while the SP sequencer is still in its preamble.
    XT = [persist.tile([128, S], F16, name=f"XT_{f}") for f in range(FC)]

    def load_xt(m, f, eng, after=None):
        d = eng.dma_start(out=XT[f][:, 512 * m:512 * (m + 1)],
                          in_=xt_d[128 * f:128 * (f + 1), 512 * m:512 * (m + 1)])
        if after is not None:
            tile.add_dep_helper(d.ins, after.ins,
                                reason="XT prefetch yields fabric to weights")
        return d

    for f in range(3):
        load_xt(0, f, nc.gpsimd)
    for f in range(3, FC):
        load_xt(0, f, nc.sync)

    # Weights already packed on host: wall = [wq16 | wk16 | wv16], each
    # [128, 1152] f-major (cols 192*f + d); qk2 = [wq_h2 | wk_h2] per f.
    wall = const.tile([128, 3 * 6 * 192], F16)
    wall_dma = nc.gpsimd.dma_start(out=wall, in_=w_d[:, :])
    w16 = {t: wall[:, 1152 * t:1152 * (t + 1)] for t in range(3)}
    qk2t = const.tile([128, 6 * 128], F16)
    nc.gpsimd.dma_start(out=qk2t, in_=qk2_d[:, :])
    qk2 = [qk2t[:, 128 * f:128 * (f + 1)] for f in range(FC)]

    mask_t = const.tile([128, KC], F32)  # mask[128*i + p] at [p, i]
    nc.gpsimd.dma_start(out=mask_t, in_=m_d[:].rearrange("(i p) -> p i", p=128))
    expm = const.tile([128, KC], F32)    # exp(mask), per k position
    nc.scalar.activation(expm, mask_t, EXP)  # early: also triggers table load

    bias_pair = []
    for t in range(2):
        bp = const.tile([128, 1], F32, name=f"bias_pair_{t}")
        nc.gpsimd.dma_start(out=bp, in_=b_ds[t][0:128].rearrange("(p o) -> p o", o=1))
        bias_pair.append(bp)
    bias_solo = []
    for t in range(2):
        bs = const.tile([64, 1], F32, name=f"bias_solo_{t}")
        nc.gpsimd.dma_start(out=bs, in_=b_ds[t][128:192].rearrange("(p o) -> p o", o=1))
        bias_solo.append(bs)
    # bv broadcast across partitions: V is built in [k, d] layout so its bias
    # varies along the free dim.
    bveq = b_ds[2][:]
    bcv = const.tile([128, 192], F32)
    nc.gpsimd.dma_start(
        out=bcv,
        in_=bass.AP(tensor=bveq.tensor, offset=bveq.offset, ap=[[0, 128], [1, 192]]))

    # Remaining X blocks: sync queue, block-major so block m lands before
    # block m+1.  Block 1 yields the DMA fabric to the weights (needed for
    # the first projections) by waiting on the wall DMA.
    for m in range(1, 4):
        for f in range(FC):
            load_xt(m, f, nc.sync, after=wall_dma if m == 1 else None)

    # --- persistent projection outputs --------------------------------------
    # QT2/KT2: [128, 2048] fp16, rows 0:64 = head0, 64:128 = head1
    # QTs/KTs: [128, 2048] fp16, head2 duplicated into both partition halves
    QT2 = persist.tile([128, S], F16)
    KT2 = persist.tile([128, S], F16)
    QTs = persist.tile([128, S], F16)
    KTs = persist.tile([128, S], F16)
    # V[kc] layout: [V_h0(64) | e | V_h1(64) | e | V_h2(64) | e], e = exp(m_k)
    V = [persist.tile([128, 195], F16, name=f"V_{kc}") for kc in range(KC)]

    out_tiles = [outp.tile([128, 192], F32, name=f"o_{u}") for u in range(16)]
    out_written = [0] * 16

    def proj_pair(t, dst_pair, m):
        cols = slice(512 * m, 512 * (m + 1))
        pp = ps.tile([128, 512], F32, name=f"proj_{t}_{m}_p", tag="sm", bufs=4)
        for f in range(FC):
            nc.tensor.matmul(pp, w16[t][:, 192 * f:192 * f + 128],
                             XT[f][:, cols], start=(f == 0), stop=(f == FC - 1))
        nc.vector.tensor_scalar_add(out=dst_pair[:, cols], in0=pp,
                                    scalar1=bias_pair[t])

    def proj_qk2(m):
        """Merged head-2 projections of Q and K: one M=128 stationary
        [wq_h2 | wk_h2], output partitions 0:64 = Q head2, 64:128 = K head2."""
        cols = slice(512 * m, 512 * (m + 1))
        sp = ps.tile([128, 512], F32, name=f"proj_s_{m}", tag="sm", bufs=4)
        for f in range(FC):
            nc.tensor.matmul(sp, qk2[f], XT[f][:, cols],
                             start=(f == 0), stop=(f == FC - 1))
        nc.vector.tensor_scalar_add(out=QTs[0:64, cols], in0=sp[0:64],
                                    scalar1=bias_solo[0])
        nc.vector.tensor_scalar_add(out=KTs[64:128, cols], in0=sp[64:128],
                                    scalar1=bias_solo[1])
        # duplicate head2 into the other partition half for row tiling
        nc.sync.dma_start(out=QTs[64:128, cols], in_=QTs[0:64, cols])
        nc.sync.dma_start(out=KTs[0:64, cols], in_=KTs[64:128, cols])

    def v_direct(kc):
        """V[kc] = (X.T chunk).T @ Wv directly in [k, d] layout, + bias,
        scaled by exp(mask), with the e column appended per head."""
        vp = ps.tile([128, 192], F32, name=f"vp_{kc}", tag="sm", bufs=4)
        for f in range(FC):
            nc.tensor.matmul(vp, XT[f][:, 128 * kc:128 * (kc + 1)],
                             w16[2][:, 192 * f:192 * (f + 1)],
                             start=(f == 0), stop=(f == FC - 1))
        v3 = bass.AP(tensor=V[kc].tensor, offset=V[kc].offset,
                     ap=[V[kc].ap[0], [65, NHL], [1, 64]])
        vp3 = bass.AP(tensor=vp.tensor, offset=vp.offset,
                      ap=[vp.ap[0], [64, NHL], [1, 64]])
        b3 = bass.AP(tensor=bcv.tensor, offset=bcv.offset,
                     ap=[bcv.ap[0], [64, NHL], [1, 64]])
        nc.vector.tensor_add(out=v3, in0=vp3, in1=b3)
        ecol = bass.AP(tensor=V[kc].tensor, offset=V[kc].offset + 64,
                       ap=[V[kc].ap[0], [65, NHL]])
        if zero_mask:
            nc.gpsimd.memset(ecol, 1.0)
        else:
            nc.vector.tensor_scalar_mul(out=v3, in0=v3,
                                        scalar1=expm[:, kc:kc + 1])
            esrc = bass.AP(tensor=expm.tensor, offset=expm.offset + kc,
                           ap=[expm.ap[0], [0, NHL]])
            nc.vector.tensor_copy(out=ecol, in_=esrc)

    def scores_group(streams, g):
        """Score matmuls + exp for one k-chunk, BOTH streams in one tile.

        streams: [(head, J, prow), (head, J, prow)] with prow 0 and 64.
        The two streams' N=256 half-matmuls alternate so the PE runs rows
        0-63 and 64-127 concurrently; a single exp drains the whole
        [128, 1024] tile so all four matmuls are gated by one PSUM-free
        event.  Returns the shared probs tile."""
        kc, _ = GROUPS[g]
        sc = ps.tile([128, 2 * QB], F32,
                     name=f"sc_{streams[0][0]}_{streams[0][1]}_{g}",
                     tag="sc", bufs=2)
        prev_mm = None
        for h in range(2):
            for i, (head, J, prow) in enumerate(streams):
                KT = KT2 if head < 2 else KTs
                QT = QT2 if head < 2 else QTs
                mm = nc.tensor.matmul(
                    sc[:, QB * i + 256 * h:QB * i + 256 * (h + 1)],
                    KT[prow:prow + 64, 128 * kc:128 * (kc + 1)],
                    QT[prow:prow + 64,
                       QB * J + 256 * h:QB * J + 256 * (h + 1)],
                    start=True, stop=True)
                if i == 1 and prev_mm is not None:
                    tile.add_dep_helper(mm.ins, prev_mm.ins, sync=False,
                                        reason="score pair adjacency")
                prev_mm = mm if i == 0 else None
        pt = probs_pool.tile([128, 2 * QB], F16,
                             name=f"pb_{streams[0][0]}_{streams[0][1]}_{g}",
                             tag="probs")
        nc.scalar.activation(pt, sc, EXP, scale=0.125)
        return pt

    def ctx_chain(head, J, probs, s, off):
        """One q-sub-chunk's ctx accumulation + normalize + out.

        probs[kc] covers both streams side by side; `off` selects this
        stream's 512-col half.  One PSUM tile (= one bank) per accumulation
        chain: start=True clears has_written for the whole bank, so chains
        must not share a bank."""
        cx = ps.tile([128, 65], F32, name=f"cx_{head}_{J}_{s}", tag="sm", bufs=4)
        for kc in range(KC):
            nc.tensor.matmul(
                cx,
                probs[kc][:, off + 128 * s:off + 128 * (s + 1)],
                V[kc][:, 65 * head:65 * head + 65],
                start=(kc == 0), stop=(kc == KC - 1))
        r = small.tile([128, 1], F32, name=f"r_{head}_{J}_{s}", tag="recip")
        nc.vector.reciprocal(r, cx[:, 64:65])
        u = 4 * J + s
        nc.vector.tensor_scalar_mul(
            out=out_tiles[u][:, 64 * head:64 * (head + 1)],
            in0=cx[:, 0:64], scalar1=r)
        out_written[u] += 1
        if out_written[u] == NHL:
            nc.sync.dma_start(out=out_d[128 * u:128 * (u + 1), :],
                              in_=out_tiles[u])

    # --- emission ------------------------------------------------------------
    # Score groups for ALL steps are spread across the projection m-blocks as
    # soon as their K/Q column blocks exist, so ScalarE's exp work overlaps the
    # whole projection phase.  A cap on un-consumed probs tiles bounds SBUF.
    all_steps = [
        ([(0, 0, 0), (1, 0, 64)], 0),
        ([(0, 1, 0), (1, 1, 64)], 1),
        ([(2, 0, 0), (2, 1, 64)], 1),
        ([(0, 2, 0), (1, 2, 64)], 2),
        ([(0, 3, 0), (1, 3, 64)], 3),
        ([(2, 2, 0), (2, 3, 64)], 3),
    ]
    units = [(si, g) for si in range(len(all_steps)) for g in range(len(GROUPS))]
    emitted = set()
    step_probs = {si: [[None] * len(GROUPS) for _ in range(2)]
                  for si in range(len(all_steps))}
    state = {"inflight": 0}
    CAP = 54  # max un-consumed score groups (one probs tile each)

    def emit_unit(si, g):
        streams, _ = all_steps[si]
        pt = scores_group(streams, g)
        step_probs[si][0][g] = pt
        step_probs[si][1][g] = pt
        emitted.add((si, g))
        state["inflight"] += 1

    def try_emit(q_m, k_m, budget, pair_only=False):
        for (si, g) in units:
            if budget <= 0 or state["inflight"] >= CAP:
                return
            if (si, g) in emitted:
                continue
            if pair_only and all_steps[si][0][0][0] == 2:
                continue
            k0, kn = GROUPS[g]
            if all_steps[si][1] <= q_m and (k0 + kn - 1) // 4 <= k_m:
                emit_unit(si, g)
                budget -= 1

    for m in range(4):
        proj_pair(1, KT2, m)   # K first: scores need all of K
        try_emit(m - 1, m - 1, 6)
        proj_pair(0, QT2, m)
        try_emit(m, m, 6, pair_only=True)
        proj_qk2(m)
        try_emit(m, m, 6)
        # V blocks trail by two so the PE never waits on the WV DMA; each
        # v_direct is followed by a score-group emission slot so ScalarE
        # never starves during these stretches.
        if m >= 2:
            for kc in range(4 * (m - 2), 4 * (m - 2) + 4):
                v_direct(kc)
                try_emit(m, m, 2)
        try_emit(m, m, 6)

    for kc in range(8, 16):
        v_direct(kc)
        try_emit(3, 3, 2)

    # Steady state: remaining score groups interleaved with ctx chains of
    # completed steps, so the PE's ctx work overlaps ScalarE's exp work.
    # Chains are spread every other group so the PE filler is even.
    pending = []
    for si, (streams, _) in enumerate(all_steps):
        for g in range(len(GROUPS)):
            if (si, g) not in emitted:
                if pending and g % 2 == 0:
                    ctx_chain(*pending.pop(0))
                emit_unit(si, g)
        for s in range(4):
            for i in range(2):
                pending.append((streams[i][0], streams[i][1],
                                step_probs[si][i], s, QB * i))
        state["inflight"] -= len(GROUPS)
    while pending:
        ctx_chain(*pending.pop(0))

    for p in (ps, outp, small, probs_pool, persist, const):
        p.release()


_NC_CACHE = {}


def _get_nc(zero_mask: bool):
    if zero_mask not in _NC_CACHE:
        _NC_CACHE[zero_mask] = _build_kernel(zero_mask)
    return _NC_CACHE[zero_mask]


def _pack_w(Wq, Wk, Wv, cols):
    """Host-side packing of the per-core weight slices into the on-chip
    layouts: wall [128, 3*1152] fp16 (per tensor, f-major cols 192f+d) and
    qk2 [128, 6*128] fp16 ([wq_h2 | wk_h2] per f chunk)."""
    packed = []
    halves = []
    for W in (Wq, Wk, Wv):
        w = np.ascontiguousarray(W[:, cols]).astype(np.float16)  # [768, 192]
        wf = w.reshape(6, 128, 192).transpose(1, 0, 2).reshape(128, 1152)
        packed.append(wf)
        halves.append(w.reshape(6, 128, 192)[:, :, 128:192])  # [6, 128, 64]
    wall = np.concatenate(packed, axis=1)                     # [128, 3456]
    qk2 = np.concatenate(
        [np.concatenate([halves[0][f], halves[1][f]], axis=1)  # [128, 128]
         for f in range(6)], axis=1)                           # [128, 768]
    return np.ascontiguousarray(wall), np.ascontiguousarray(qk2)


def kernel(hidden_states, attention_mask, Wq, bq, Wk, bk, Wv, bv, **run_kw):
    hidden_states = np.asarray(hidden_states, dtype=np.float32)
    attention_mask = np.asarray(attention_mask, dtype=np.float32)
    Wq, Wk, Wv = (np.asarray(a, dtype=np.float32) for a in (Wq, Wk, Wv))
    bq, bk, bv = (np.asarray(a, dtype=np.float32) for a in (bq, bk, bv))

    zero_mask = bool(np.all(attention_mask == 0.0))
    nc = _get_nc(zero_mask)
    xts = [np.ascontiguousarray(hidden_states[b].T.astype(np.float16))
           for b in range(2)]
    walls = {}
    in_maps = []
    for c in range(8):
        b, g = c // 4, c % 4
        cols = slice(192 * g, 192 * (g + 1))
        if g not in walls:
            walls[g] = _pack_w(Wq, Wk, Wv, cols)
        wall, qk2 = walls[g]
        in_maps.append({
            "xt": xts[b],
            "wall": wall,
            "qk2": qk2,
            "bq": np.ascontiguousarray(bq[cols]),
            "bk": np.ascontiguousarray(bk[cols]),
            "bv": np.ascontiguousarray(bv[cols]),
            "mask": np.ascontiguousarray(
                np.broadcast_to(attention_mask[b, 0, 0], (S,))),
        })
    res = run_bass_kernel_spmd(nc, in_maps, list(range(8)), **run_kw)
    out = np.empty((2, S, DM), dtype=np.float32)
    for c in range(8):
        b, g = c // 4, c % 4
        out[b, :, 192 * g:192 * (g + 1)] = res.results[c]["out"]
    if run_kw:
        return out, res
    return out


# revision 30
# speedup vs baseline: 1.1952x; 1.1952x over previous
"""BERT self-attention Bass/Tile kernel for 8 Trainium2 NeuronCores.

Problem: hidden [2, 2048, 768], 12 heads x 64 dim, additive mask [2,1,1,2048].
Sharding: batch x head-group. Core c handles batch b = c // 4 and global heads
3*(c%4) .. 3*(c%4)+2 (columns 192*(c%4) .. +192 of Wq/Wk/Wv).  Each core
computes its 3 heads' full attention locally; outputs are concatenated on the
host (no cross-device communication).

Host-side prep (part of sharding): X is passed pre-transposed and pre-cast to
fp16 ([768, 2048] per batch), and the weight slices are packed into their
on-chip fp16 layouts.  The device then runs pure matmul pipeline with no
PE transposes and no DVE casts:

  XT[f] [128, 2048] fp16  <- DMA (m-block pieces, f0-2 via SWDGE early)
  Q_T/K_T = W.T @ X_T     (heads 0/1 packed M=128; head-2 of Q and K merged
                           into one M=128 stationary [wq_h2|wk_h2])
  V[k, d] = X_T.T @ Wv    directly in [k, d] layout (XT chunk stationary);
                           bias via partition-broadcast tile on DVE
  scores_T[k,q] = K_T.T @ Q_T  (K=64 contraction; the two 64-row streams run
                                concurrently in the PE's row halves)
  probs = exp(scores/8) via ScalarE: ONE [128, 1024] PSUM tile and ONE exp
    per (k-chunk, step) covering both streams, so all four score matmuls are
    released by a single PSUM-free event (keeps pair concurrency intact under
    just-in-time recycling).
  V[kc] layout [V_h0|e|V_h1|e|V_h2|e] where e = exp(mask_k) column
  ctx_aug[q, 65] = probs_T.T @ V_aug  (col 64 = softmax denominator)
  out[q, d] = ctx[:, :64] * (1 / ctx[:, 64])   -> DMA to DRAM

The additive mask is folded into V: exp(s + m_k) = exp(s) * exp(m_k), so both
the numerator and the denominator column of V_aug are pre-scaled by exp(m_k).
When the mask is all zeros (the common case) that scale is skipped and the
denominator column is just memset to 1.
"""

import numpy as np

import concourse.bass as bass
import concourse.tile as tile
from concourse import bacc, mybir
from concourse.bass_utils import run_bass_kernel_spmd

F32 = mybir.dt.float32
F16 = mybir.dt.float16
EXP = mybir.ActivationFunctionType.Exp

S = 2048           # sequence length
DM = 768           # model dim
DH = 64            # head dim
NHL = 3            # local heads per core
FC = DM // 128     # 6 f-chunks (contraction for projections)
KC = S // 128      # 16 k-chunks
QB = 512           # q block width for score matmuls
NQB = S // QB      # 4 q blocks
# One group = one k-chunk and BOTH row-tiled streams in a single [128, 1024]
# PSUM tile (2 banks) drained by a single exp.
GROUPS = [(i, 1) for i in range(16)]


def _build_kernel(zero_mask: bool) -> bass.Bass:
    nc = bacc.Bacc()

    xt_d = nc.declare_dram_parameter("xt", [DM, S], F16, isOutput=False)
    w_d = nc.declare_dram_parameter("wall", [128, 3 * 6 * 192], F16,
                                    isOutput=False)
    qk2_d = nc.declare_dram_parameter("qk2", [128, 6 * 128], F16,
                                      isOutput=False)
    bq_d = nc.declare_dram_parameter("bq", [192], F32, isOutput=False)
    bk_d = nc.declare_dram_parameter("bk", [192], F32, isOutput=False)
    bv_d = nc.declare_dram_parameter("bv", [192], F32, isOutput=False)
    m_d = nc.declare_dram_parameter("mask", [S], F32, isOutput=False)
    out_d = nc.declare_dram_parameter("out", [S, 192], F32, isOutput=True)

    with tile.TileContext(nc) as tc:
        _attention(tc, xt_d, w_d, qk2_d, (bq_d, bk_d, bv_d), m_d, out_d,
                   zero_mask)
    nc.compile()
    return nc


def _attention(tc, xt_d, w_d, qk2_d, b_ds, m_d, out_d, zero_mask):
    nc = tc.nc

    const = tc.alloc_tile_pool(name="const", bufs=1)
    persist = tc.alloc_tile_pool(name="persist", bufs=1)
    probs_pool = tc.alloc_tile_pool(name="probs", bufs=58)
    small = tc.alloc_tile_pool(name="small", bufs=4)
    outp = tc.alloc_tile_pool(name="outp", bufs=1)
    ps = tc.alloc_tile_pool(name="ps", bufs=2, space="PSUM")

    # XT[f]: X.T rows 128f..128(f+1), fp16 [128, 2048], loaded in m-block
    # pieces so block 0 lands first.  f0-2 of block 0 go out on gpsimd whose
    # SWDGE issues at ~2.5us, while the SP sequencer is still in its preamble.
    XT = [persist.tile([128, S], F16, name=f"XT_{f}") for f in range(FC)]

    def load_xt(m, f, eng, after=None):
        d = eng.dma_start(out=XT[f][:, 512 * m:512 * (m + 1)],
                          in_=xt_d[128 * f:128 * (f + 1), 512 * m:512 * (m + 1)])
        if after is not None:
            tile.add_dep_helper(d.ins, after.ins,
                                reason="XT prefetch yields fabric to weights")
        return d

    # Weights already packed on host: wall = [wq16 | wk16 | wv16], each
    # [128, 1152] f-major (cols 192*f + d); qk2 = [wq_h2 | wk_h2] per f.
    # gpsimd issue order is the startup priority order: wk, X0 f0-2, wq,
    # qk2, wv — the K projection's inputs land first.
    wall = const.tile([128, 3 * 6 * 192], F16)
    w16 = {t: wall[:, 1152 * t:1152 * (t + 1)] for t in range(3)}

    def load_w(t):
        nc.gpsimd.dma_start(out=wall[:, 1152 * t:1152 * (t + 1)],
                            in_=w_d[:, 1152 * t:1152 * (t + 1)])

    load_w(1)
    for f in range(3):
        load_xt(0, f, nc.gpsimd)
    for f in range(3, FC):
        load_xt(0, f, nc.sync)
    load_w(0)
    qk2t = const.tile([128, 6 * 128], F16)
    nc.gpsimd.dma_start(out=qk2t, in_=qk2_d[:, :])
    qk2 = [qk2t[:, 128 * f:128 * (f + 1)] for f in range(FC)]
    load_w(2)

    mask_t = const.tile([128, KC], F32)  # mask[128*i + p] at [p, i]
    nc.gpsimd.dma_start(out=mask_t, in_=m_d[:].rearrange("(i p) -> p i", p=128))
    expm = const.tile([128, KC], F32)    # exp(mask), per k position
    nc.scalar.activation(expm, mask_t, EXP)  # early: also triggers table load

    bias_pair = []
    for t in range(2):
        bp = const.tile([128, 1], F32, name=f"bias_pair_{t}")
        nc.gpsimd.dma_start(out=bp, in_=b_ds[t][0:128].rearrange("(p o) -> p o", o=1))
        bias_pair.append(bp)
    bias_solo = []
    for t in range(2):
        bs = const.tile([64, 1], F32, name=f"bias_solo_{t}")
        nc.gpsimd.dma_start(out=bs, in_=b_ds[t][128:192].rearrange("(p o) -> p o", o=1))
        bias_solo.append(bs)
    # bv broadcast across partitions: V is built in [k, d] layout so its bias
    # varies along the free dim.
    bveq = b_ds[2][:]
    bcv = const.tile([128, 192], F32)
    nc.gpsimd.dma_start(
        out=bcv,
        in_=bass.AP(tensor=bveq.tensor, offset=bveq.offset, ap=[[0, 128], [1, 192]]))

    # Remaining X blocks: sync queue, block-major so block m lands before
    # block m+1.
    for m in range(1, 4):
        for f in range(FC):
            load_xt(m, f, nc.sync)

    # --- persistent projection outputs --------------------------------------
    # QT2/KT2: [128, 2048] fp16, rows 0:64 = head0, 64:128 = head1
    # QTs/KTs: [128, 2048] fp16, head2 duplicated into both partition halves
    QT2 = persist.tile([128, S], F16)
    KT2 = persist.tile([128, S], F16)
    QTs = persist.tile([128, S], F16)
    KTs = persist.tile([128, S], F16)
    # V[kc] layout: [V_h0(64) | e | V_h1(64) | e | V_h2(64) | e], e = exp(m_k)
    V = [persist.tile([128, 195], F16, name=f"V_{kc}") for kc in range(KC)]

    out_tiles = [outp.tile([128, 192], F32, name=f"o_{u}") for u in range(16)]
    out_written = [0] * 16

    def proj_pair(t, dst_pair, m):
        cols = slice(512 * m, 512 * (m + 1))
        pp = ps.tile([128, 512], F32, name=f"proj_{t}_{m}_p", tag="sm", bufs=4)
        for f in range(FC):
            nc.tensor.matmul(pp, w16[t][:, 192 * f:192 * f + 128],
                             XT[f][:, cols], start=(f == 0), stop=(f == FC - 1))
        nc.vector.tensor_scalar_add(out=dst_pair[:, cols], in0=pp,
                                    scalar1=bias_pair[t])

    def proj_qk2(m):
        """Merged head-2 projections of Q and K: one M=128 stationary
        [wq_h2 | wk_h2], output partitions 0:64 = Q head2, 64:128 = K head2."""
        cols = slice(512 * m, 512 * (m + 1))
        sp = ps.tile([128, 512], F32, name=f"proj_s_{m}", tag="sm", bufs=4)
        for f in range(FC):
            nc.tensor.matmul(sp, qk2[f], XT[f][:, cols],
                             start=(f == 0), stop=(f == FC - 1))
        nc.vector.tensor_scalar_add(out=QTs[0:64, cols], in0=sp[0:64],
                                    scalar1=bias_solo[0])
        nc.vector.tensor_scalar_add(out=KTs[64:128, cols], in0=sp[64:128],
                                    scalar1=bias_solo[1])
        # duplicate head2 into the other partition half for row tiling
        nc.sync.dma_start(out=QTs[64:128, cols], in_=QTs[0:64, cols])
        nc.sync.dma_start(out=KTs[0:64, cols], in_=KTs[64:128, cols])

    def v_direct(kc):
        """V[kc] = (X.T chunk).T @ Wv directly in [k, d] layout, + bias,
        scaled by exp(mask), with the e column appended per head."""
        vp = ps.tile([128, 192], F32, name=f"vp_{kc}", tag="sm", bufs=4)
        for f in range(FC):
            nc.tensor.matmul(vp, XT[f][:, 128 * kc:128 * (kc + 1)],
                             w16[2][:, 192 * f:192 * (f + 1)],
                             start=(f == 0), stop=(f == FC - 1))
        v3 = bass.AP(tensor=V[kc].tensor, offset=V[kc].offset,
                     ap=[V[kc].ap[0], [65, NHL], [1, 64]])
        vp3 = bass.AP(tensor=vp.tensor, offset=vp.offset,
                      ap=[vp.ap[0], [64, NHL], [1, 64]])
        b3 = bass.AP(tensor=bcv.tensor, offset=bcv.offset,
                     ap=[bcv.ap[0], [64, NHL], [1, 64]])
        nc.vector.tensor_add(out=v3, in0=vp3, in1=b3)
        ecol = bass.AP(tensor=V[kc].tensor, offset=V[kc].offset + 64,
                       ap=[V[kc].ap[0], [65, NHL]])
        if zero_mask:
            nc.gpsimd.memset(ecol, 1.0)
        else:
            nc.vector.tensor_scalar_mul(out=v3, in0=v3,
                                        scalar1=expm[:, kc:kc + 1])
            esrc = bass.AP(tensor=expm.tensor, offset=expm.offset + kc,
                           ap=[expm.ap[0], [0, NHL]])
            nc.vector.tensor_copy(out=ecol, in_=esrc)

    def scores_group(streams, g):
        """Score matmuls + exp for one k-chunk, BOTH streams in one tile.

        streams: [(head, J, prow), (head, J, prow)] with prow 0 and 64.
        The two streams' N=256 half-matmuls alternate so the PE runs rows
        0-63 and 64-127 concurrently; a single exp drains the whole
        [128, 1024] tile so all four matmuls are gated by one PSUM-free
        event.  Returns the shared probs tile."""
        kc, _ = GROUPS[g]
        sc = ps.tile([128, 2 * QB], F32,
                     name=f"sc_{streams[0][0]}_{streams[0][1]}_{g}",
                     tag="sc", bufs=2)
        prev_mm = None
        for h in range(2):
            for i, (head, J, prow) in enumerate(streams):
                KT = KT2 if head < 2 else KTs
                QT = QT2 if head < 2 else QTs
                mm = nc.tensor.matmul(
                    sc[:, QB * i + 256 * h:QB * i + 256 * (h + 1)],
                    KT[prow:prow + 64, 128 * kc:128 * (kc + 1)],
                    QT[prow:prow + 64,
                       QB * J + 256 * h:QB * J + 256 * (h + 1)],
                    start=True, stop=True)
                if i == 1 and prev_mm is not None:
                    tile.add_dep_helper(mm.ins, prev_mm.ins, sync=False,
                                        reason="score pair adjacency")
                prev_mm = mm if i == 0 else None
        pt = probs_pool.tile([128, 2 * QB], F16,
                             name=f"pb_{streams[0][0]}_{streams[0][1]}_{g}",
                             tag="probs")
        nc.scalar.activation(pt, sc, EXP, scale=0.125)
        return pt

    def ctx_chain(head, J, probs, s, off):
        """One q-sub-chunk's ctx accumulation + normalize + out.

        probs[kc] covers both streams side by side; `off` selects this
        stream's 512-col half.  One PSUM tile (= one bank) per accumulation
        chain: start=True clears has_written for the whole bank, so chains
        must not share a bank."""
        cx = ps.tile([128, 65], F32, name=f"cx_{head}_{J}_{s}", tag="sm", bufs=4)
        for kc in range(KC):
            nc.tensor.matmul(
                cx,
                probs[kc][:, off + 128 * s:off + 128 * (s + 1)],
                V[kc][:, 65 * head:65 * head + 65],
                start=(kc == 0), stop=(kc == KC - 1))
        r = small.tile([128, 1], F32, name=f"r_{head}_{J}_{s}", tag="recip")
        nc.vector.reciprocal(r, cx[:, 64:65])
        u = 4 * J + s
        nc.vector.tensor_scalar_mul(
            out=out_tiles[u][:, 64 * head:64 * (head + 1)],
            in0=cx[:, 0:64], scalar1=r)
        out_written[u] += 1
        if out_written[u] == NHL:
            nc.sync.dma_start(out=out_d[128 * u:128 * (u + 1), :],
                              in_=out_tiles[u])

    # --- emission ------------------------------------------------------------
    # Score groups for ALL steps are spread across the projection m-blocks as
    # soon as their K/Q column blocks exist, so ScalarE's exp work overlaps the
    # whole projection phase.  A cap on un-consumed probs tiles bounds SBUF.
    all_steps = [
        ([(0, 0, 0), (1, 0, 64)], 0),
        ([(0, 1, 0), (1, 1, 64)], 1),
        ([(2, 0, 0), (2, 1, 64)], 1),
        ([(0, 2, 0), (1, 2, 64)], 2),
        ([(0, 3, 0), (1, 3, 64)], 3),
        ([(2, 2, 0), (2, 3, 64)], 3),
    ]
    units = [(si, g) for si in range(len(all_steps)) for g in range(len(GROUPS))]
    emitted = set()
    step_probs = {si: [[None] * len(GROUPS) for _ in range(2)]
                  for si in range(len(all_steps))}
    state = {"inflight": 0}
    CAP = 54  # max un-consumed score groups (one probs tile each)

    def emit_unit(si, g):
        streams, _ = all_steps[si]
        pt = scores_group(streams, g)
        step_probs[si][0][g] = pt
        step_probs[si][1][g] = pt
        emitted.add((si, g))
        state["inflight"] += 1

    def try_emit(q_m, k_m, budget, pair_only=False):
        for (si, g) in units:
            if budget <= 0 or state["inflight"] >= CAP:
                return
            if (si, g) in emitted:
                continue
            if pair_only and all_steps[si][0][0][0] == 2:
                continue
            k0, kn = GROUPS[g]
            if all_steps[si][1] <= q_m and (k0 + kn - 1) // 4 <= k_m:
                emit_unit(si, g)
                budget -= 1

    for m in range(4):
        proj_pair(1, KT2, m)   # K first: scores need all of K
        try_emit(m - 1, m - 1, 6)
        proj_pair(0, QT2, m)
        try_emit(m, m, 6, pair_only=True)
        proj_qk2(m)
        try_emit(m, m, 6)
        # V blocks trail by two so the PE never waits on the WV DMA; each
        # v_direct is followed by a score-group emission slot so ScalarE
        # never starves during these stretches.
        if m >= 2:
            for kc in range(4 * (m - 2), 4 * (m - 2) + 4):
                v_direct(kc)
                try_emit(m, m, 2)
        try_emit(m, m, 6)

    for kc in range(8, 16):
        v_direct(kc)
        try_emit(3, 3, 2)

    # Steady state: remaining score groups interleaved with ctx chains of
    # completed steps, so the PE's ctx work overlaps ScalarE's exp work.
    # Chains are spread every other group so the PE filler is even.
    pending = []
    for si, (streams, _) in enumerate(all_steps):
        for g in range(len(GROUPS)):
            if (si, g) not in emitted:
                if pending and g % 2 == 0:
                    ctx_chain(*pending.pop(0))
                emit_unit(si, g)
        for s in range(4):
            for i in range(2):
                pending.append((streams[i][0], streams[i][1],
                                step_probs[si][i], s, QB * i))
        state["inflight"] -= len(GROUPS)
    while pending:
        ctx_chain(*pending.pop(0))

    for p in (ps, outp, small, probs_pool, persist, const):
        p.release()


_NC_CACHE = {}


def _get_nc(zero_mask: bool):
    if zero_mask not in _NC_CACHE:
        _NC_CACHE[zero_mask] = _build_kernel(zero_mask)
    return _NC_CACHE[zero_mask]


def _pack_w(Wq, Wk, Wv, cols):
    """Host-side packing of the per-core weight slices into the on-chip
    layouts: wall [128, 3*1152] fp16 (per tensor, f-major cols 192f+d) and
    qk2 [128, 6*128] fp16 ([wq_h2 | wk_h2] per f chunk)."""
    packed = []
    halves = []
    for W in (Wq, Wk, Wv):
        w = np.ascontiguousarray(W[:, cols]).astype(np.float16)  # [768, 192]
        wf = w.reshape(6, 128, 192).transpose(1, 0, 2).reshape(128, 1152)
        packed.append(wf)
        halves.append(w.reshape(6, 128, 192)[:, :, 128:192])  # [6, 128, 64]
    wall = np.concatenate(packed, axis=1)                     # [128, 3456]
    qk2 = np.concatenate(
        [np.concatenate([halves[0][f], halves[1][f]], axis=1)  # [128, 128]
         for f in range(6)], axis=1)                           # [128, 768]
    return np.ascontiguousarray(wall), np.ascontiguousarray(qk2)


def kernel(hidden_states, attention_mask, Wq, bq, Wk, bk, Wv, bv, **run_kw):
    hidden_states = np.asarray(hidden_states, dtype=np.float32)
    attention_mask = np.asarray(attention_mask, dtype=np.float32)
    Wq, Wk, Wv = (np.asarray(a, dtype=np.float32) for a in (Wq, Wk, Wv))
    bq, bk, bv = (np.asarray(a, dtype=np.float32) for a in (bq, bk, bv))

    zero_mask = bool(np.all(attention_mask == 0.0))
    nc = _get_nc(zero_mask)
    xts = [np.ascontiguousarray(hidden_states[b].T.astype(np.float16))
           for b in range(2)]
    walls = {}
    in_maps = []
    for c in range(8):
        b, g = c // 4, c % 4
        cols = slice(192 * g, 192 * (g + 1))
        if g not in walls:
            walls[g] = _pack_w(Wq, Wk, Wv, cols)
        wall, qk2 = walls[g]
        in_maps.append({
            "xt": xts[b],
            "wall": wall,
            "qk2": qk2,
            "bq": np.ascontiguousarray(bq[cols]),
            "bk": np.ascontiguousarray(bk[cols]),
            "bv": np.ascontiguousarray(bv[cols]),
            "mask": np.ascontiguousarray(
                np.broadcast_to(attention_mask[b, 0, 0], (S,))),
        })
    res = run_bass_kernel_spmd(nc, in_maps, list(range(8)), **run_kw)
    out = np.empty((2, S, DM), dtype=np.float32)
    for c in range(8):
        b, g = c // 4, c % 4
        out[b, :, 192 * g:192 * (g + 1)] = res.results[c]["out"]
    if run_kw:
        return out, res
    return out
